# revision 32
# baseline (speedup 1.0000x reference)
"""Trainium2 Bass kernel for nn_NodeNet: GNN message passing + 12-qubit TTN circuit.

Math: the reference's statevector circuit contracts exactly to per-node
Bloch-vector chains (every CNOT block keeps only its target wire; the
measurement is <Z_9>; the circuit is a tree so alive wires stay in
product states). Per node the whole circuit is ~60 scalar ops.

The relation matrices Ri/Ro are one-hot per edge column (built with
.at[idx, arange(E)].set(1)), so the graph phase is a gather + weighted
scatter-add:  mi[n] = sum_{e: idx_i[e]=n} e[e] * X[idx_o[e]]  (mo swaps
the roles of idx_i/idx_o).

Sharding: edges are routed to the core that owns their DESTINATION node
chunk (128 nodes per core), separately for mi (by idx_i) and mo (by
idx_o).  Each core then computes mi/mo for its own nodes only -- no
cross-core reduction at all.  Within a core, edges are sorted by SOURCE
chunk so the gather one-hot only ever contracts over one 128-row X
chunk: layout = 8 full 128-col segments (first 128 edges per src chunk)
plus a 32-aligned overflow region.

Stage 1 (gather):  stationary = one-hot [128 src, seg cols], moving =
X chunk hi/lo bf16 [128, 8] -> PSUM [e-part, 8] per e-chunk.
Stage 2: hi+lo merge, scale by e, re-split to bf16 hi/lo (vector ops).
Stage 3 (scatter): stationary = dst one-hot [128 e, 128 n], moving =
beo [128, 8], accumulated over e-chunks -> PSUM [128 n, 8] which is
exactly the circuit's angle layout (features pre-permuted on host).

Precision: one-hots are exact in bf16; X and beo are carried as bf16
high+low splits (fp32-grade accuracy); all accumulation in fp32 PSUM.
"""

import hashlib

import ml_dtypes
import numpy as np

import bass_rust
import concourse.bass as bass
import concourse.mybir as mybir
import concourse.tile as tile
from concourse.bass_utils import run_bass_kernel_spmd

F32 = mybir.dt.float32
BF16 = mybir.dt.bfloat16
N_CORES = 8
N, E, D = 1024, 8192, 4
P = 128                  # partitions / nodes per core
NCH = N // P             # 8 node chunks

_BLOCKS = [(0, 1, (0, 1)), (2, 3, (3, 2)), (4, 5, (4, 5)), (6, 7, (7, 6)),
           (8, 9, (8, 9)), (10, 11, (11, 10)), (1, 2, (1, 2)), (5, 6, (6, 5)),
           (9, 10, (10, 9)), (2, 5, (2, 5)), (5, 9, (5, 9))]

# M-angle column layout (see baseline derivation):
# m cols 0:6 = layer-A target wires [w1, w6, w10, w2, w5, w9]
# m cols 6:12 = layer-A control wires [w0, w7, w11, w3, w4, w8]
# stride-3 view: col t of group g: {0,3,6,9}=mi, {1,4,7,10}=mo, {2,5,8,11}=X
A_BLOCKS = [0, 3, 5, 1, 2, 4]     # block idx per A-target col
B_BLOCKS = [6, 7, 8]              # b-cols [w2, w5, w9] <- a-cols [w1, w6, w10]
MI_PERM = [1, 2, 0, 3]            # mi feature cols in M stride-3 order
MO_PERM = [2, 1, 3, 0]            # mo feature cols
XK_PERM = [2, 1, 3, 0]            # X columns in M stride-3 order

# ---------------------------------------------------------------------------
# Host-side circuit-constant preparation (identical to proven baseline)
# ---------------------------------------------------------------------------

_PAULI = np.array([
    [[0, 1], [1, 0]],
    [[0, -1j], [1j, 0]],
    [[1, 0], [0, -1]],
], dtype=np.complex128)


def _rot_so3(p):
    """SO(3) Bloch rotation of Rot(phi, theta, omega) = RZ(om) RY(th) RZ(phi)."""
    phi, th, om = float(p[0]), float(p[1]), float(p[2])
    c, s = np.cos(th / 2), np.sin(th / 2)
    U = np.array([
        [np.exp(-0.5j * (phi + om)) * c, -np.exp(0.5j * (phi - om)) * s],
        [np.exp(-0.5j * (phi - om)) * s, np.exp(0.5j * (phi + om)) * c],
    ])
    R = np.empty((3, 3))
    for i in range(3):
        for j in range(3):
            R[i, j] = 0.5 * np.real(
                np.trace(_PAULI[i] @ U @ _PAULI[j] @ U.conj().T))
    return R


# circuit-constants column layout (offsets into the ck segment of smalls)
CK_A = 0          # layer A target coefs [(i*6+t)*2+m], m: 0=sin, 1=cos term
CK_Z6 = 36        # layer A control row2 [(t*2+m)]
CK_B = 48         # layer B target coefs [(i*3+t)*3+j]
CK_Z3 = 75        # layer B control row2 [(t*3+j)]
CK_C18 = 84       # R18 row 2 (block 9 control rot)
CK_C19 = 87       # R19 full 3x3 [i*3+j] (block 9 target rot)
CK_C20 = 96      # R20 row 2 (block 10 control rot)
CK_C21 = 99      # R21 row 2 (block 10 target rot)
CK_W = 102


def _pack_ck(theta):
    th = np.asarray(theta, np.float64)
    R = [_rot_so3(th[3 * k:3 * k + 3]) for k in range(23)]
    ck = np.zeros(CK_W, np.float64)

    for t, bidx in enumerate(A_BLOCKS):
        w1, w2, (c, tt) = _BLOCKS[bidx]
        k1, k2 = 2 * bidx, 2 * bidx + 1
        Rc = R[k1] if c == w1 else R[k2]
        Rt = R[k1] if tt == w1 else R[k2]
        for i in range(3):
            for m, j2 in enumerate((0, 2)):
                ck[CK_A + (i * 6 + t) * 2 + m] = Rt[i, j2]
        for m, j2 in enumerate((0, 2)):
            ck[CK_Z6 + t * 2 + m] = Rc[2, j2]

    for t, bidx in enumerate(B_BLOCKS):
        w1, w2, (c, tt) = _BLOCKS[bidx]
        k1, k2 = 2 * bidx, 2 * bidx + 1
        Rc = R[k1] if c == w1 else R[k2]
        Rt = R[k1] if tt == w1 else R[k2]
        for i in range(3):
            for j in range(3):
                ck[CK_B + (i * 3 + t) * 3 + j] = Rt[i, j]
        for j in range(3):
            ck[CK_Z3 + t * 3 + j] = Rc[2, j]

    # layer C: block 9 = (2,5,(2,5)): control rot R[18] (wire2), target R[19]
    #          block 10 = (5,9,(5,9)): control rot R[20] (wire5), target R[21]
    ck[CK_C19:CK_C19 + 9] = R[19].reshape(-1)
    ck[CK_C18:CK_C18 + 3] = R[18][2]
    ck[CK_C21:CK_C21 + 3] = R[21][2]
    ck[CK_C20:CK_C20 + 3] = R[20][2]
    return ck.astype(np.float32)


# ---------------------------------------------------------------------------
# Edge routing plan (host): data-dependent but deterministic per input set
# ---------------------------------------------------------------------------


class _Plan:
    """Shared-across-cores layout: per-src-chunk full segments (128 cols)
    plus 32-aligned overflow sub-segments, and the static MM interval list.
    """

    def __init__(self, idx_dst, idx_src):
        core = idx_dst >> 7
        chunk = idx_src >> 7
        cnt = np.bincount(core * NCH + chunk, minlength=64).reshape(8, NCH)
        ow = np.maximum(cnt - P, 0).max(axis=0)
        ow = (ow + 31) // 32 * 32                  # overflow width per chunk
        base_off = [c * P for c in range(NCH)]     # full segments
        of_off = []
        pos = NCH * P
        for c in range(NCH):
            of_off.append(pos)
            pos += int(ow[c])
        s1w = (pos + P - 1) // P * P               # pad to e-chunk multiple
        self.cnt, self.ow = cnt, ow
        self.base_off, self.of_off = base_off, of_off
        self.s1w = s1w
        self.nch3 = s1w // P
        # MM intervals: (col_start, width, src_chunk), split at 128 bounds
        iv = []
        for c in range(NCH):
            iv.append((base_off[c], P, c))
            if ow[c] > 0:
                # 32-col pieces keep every PSUM base/width combo legal
                a, w = of_off[c], int(ow[c])
                while w > 0:
                    take = 32
                    iv.append((a, take, c))
                    a += take
                    w -= take
        # cover trailing dead cols (all-zero one-hot -> writes zeros)
        end = max(a + w for a, w, _ in iv)
        while end < s1w:
            iv.append((end, 32, NCH - 1))
            end += 32
        self.intervals = tuple(sorted(iv))

    def key(self):
        return (self.s1w, self.intervals)


def _route(idx_dst, idx_src, ew, plan):
    """Per-core edge placement: returns (cols[core][j]->edge or -1)."""
    core = idx_dst >> 7
    chunk = idx_src >> 7
    order = np.lexsort((chunk, core))
    place = np.full((N_CORES, plan.s1w), -1, np.int64)
    for k in range(N_CORES):
        sel = order[core[order] == k]
        ch = chunk[sel]
        for c in range(NCH):
            seg = sel[ch == c]
            nfull = min(len(seg), P)
            place[k, plan.base_off[c]:plan.base_off[c] + nfull] = seg[:nfull]
            nov = len(seg) - nfull
            if nov > 0:
                o = plan.of_off[c]
                place[k, o:o + nov] = seg[nfull:]
    return place


# ---------------------------------------------------------------------------
# Walrus workaround: this build rejects >1 sync-wait per instruction
# ---------------------------------------------------------------------------


def _split_multi_waits(nc):
    for f in nc.m.functions:
        for bb in f.blocks:
            out = []
            for inst in bb.instructions:
                si = inst.sync_info
                if si is not None and si.on_wait and len(si.on_wait) > 1:
                    waits = list(si.on_wait)
                    for i, w in enumerate(waits[:-1]):
                        out.append(mybir.InstNoOp(
                            name=f"{inst.name}_wsplit{i}",
                            engine=inst.engine,
                            ins=[], outs=[],
                            sync_info=bass_rust.SyncInfo(
                                on_wait=[w], on_update=[]),
                        ))
                    inst.sync_info = bass_rust.SyncInfo(
                        on_wait=[waits[-1]], on_update=list(si.on_update))
                out.append(inst)
            bb.instructions = out


# ---------------------------------------------------------------------------
# Device kernel
# ---------------------------------------------------------------------------

# smalls layout: [ew_i(NCH3) | ew_o(NCH3) | xk(4) | ck(CK_W)]


def _build_nc(plan_i, plan_o):
    nc = bass.Bass("TRN2", target_bir_lowering=False, num_devices=N_CORES)

    nch3_i, nch3_o = plan_i.nch3, plan_o.nch3
    sm_xk = nch3_i + nch3_o
    sm_ck = sm_xk + 4
    sm_w = sm_ck + CK_W

    s1i_d = nc.declare_dram_parameter("s1i", [P, plan_i.s1w], BF16,
                                      isOutput=False)
    s1o_d = nc.declare_dram_parameter("s1o", [P, plan_o.s1w], BF16,
                                      isOutput=False)
    t3i_d = nc.declare_dram_parameter("t3i", [P, nch3_i * P], BF16,
                                      isOutput=False)
    t3o_d = nc.declare_dram_parameter("t3o", [P, nch3_o * P], BF16,
                                      isOutput=False)
    # one combined small tensor = one DMA (descriptor count is per-DMA);
    # cols 0:32 / 32:64 hold xhl_i / xhl_o bf16 pairs bit-packed in f32
    sm_d = nc.declare_dram_parameter("smalls", [P, 64 + sm_w], F32,
                                     isOutput=False)
    # [4, 32] so the store is 4 contiguous descriptors, not 128 4-byte ones
    # (the epilogue waits on store completion; scattered stores cost ~6us)
    out = nc.declare_dram_parameter("out", [4, 32], F32, isOutput=True)

    HPI = float(np.pi / 2)
    PI = float(np.pi)
    MUL = mybir.AluOpType.mult
    ADD = mybir.AluOpType.add

    with tile.TileContext(nc) as tc:
        with (
            tc.tile_pool(name="big", bufs=1) as big,
            tc.tile_pool(name="small", bufs=1) as small,
            tc.tile_pool(name="ps1", bufs=2, space="PSUM") as ps1p,
            tc.tile_pool(name="ps3", bufs=2, space="PSUM") as ps3p,
        ):
            # ---- inputs (HWDGE queues only; SWDGE drains cost us) ------
            # order: stage-1-i gates everything, so s1_i issues first
            s1_sb, t3_sb = {}, {}
            s1_sb["i"] = big.tile([P, plan_i.s1w], BF16, name="s1_i",
                                  tag="s1_i")
            nc.sync.dma_start(s1_sb["i"][:], s1i_d[:])
            comb = small.tile([P, 64 + sm_w], F32, name="comb")
            nc.scalar.dma_start(comb[:], sm_d[:])
            s1_sb["o"] = big.tile([P, plan_o.s1w], BF16, name="s1_o",
                                  tag="s1_o")
            nc.sync.dma_start(s1_sb["o"][:], s1o_d[:])
            t3_sb["i"] = big.tile([P, plan_i.nch3 * P], BF16, name="t3_i",
                                  tag="t3_i")
            nc.scalar.dma_start(t3_sb["i"][:], t3i_d[:])
            t3_sb["o"] = big.tile([P, plan_o.nch3 * P], BF16, name="t3_o",
                                  tag="t3_o")
            nc.scalar.dma_start(t3_sb["o"][:], t3o_d[:])
            xhl = {"i": comb[:, 0:32].bitcast(BF16),
                   "o": comb[:, 32:64].bitcast(BF16)}
            sm_sb = comb[:, 64:]

            def ckc(off, n=1):
                return sm_sb[:, sm_ck + off:sm_ck + off + n]

            # preload the ACT Sin table set while DMAs stream
            warm = small.tile([P, 1], F32, name="warm")
            nc.vector.memset(warm[:], 0.0)
            nc.scalar.activation(warm[:], warm[:],
                                 mybir.ActivationFunctionType.Sin)
            tr_in = small.tile([P, 32], F32, name="tr_in")
            nc.vector.memset(tr_in[:], 0.0)
            # rne magic constant tile: adding 1.5*2^23 forces fp32 RNE to
            # integer; subtracting it back recovers rne(x) exactly
            RNE_M = float(1.5 * 2 ** 23)
            mtile = small.tile([P, 24], F32, name="mtile")
            nc.vector.memset(mtile[:], RNE_M)

            # ---- stage 1: bo[e, d(hi,lo)] = onehot_src^T @ Xsplit ------
            ps1 = {}
            for rel, plan in (("i", plan_i), ("o", plan_o)):
                ps = ps1p.tile([P, plan.nch3 * 8], F32, name=f"ps1_{rel}",
                               tag="ps1")
                for (a, w, c) in plan.intervals:
                    ec, a2 = a // P, a % P
                    nc.tensor.matmul(
                        ps[a2:a2 + w, ec * 8:ec * 8 + 8],
                        s1_sb[rel][:, a:a + w],
                        xhl[rel][:, c * 8:(c + 1) * 8],
                        start=True, stop=True,
                        tile_position=(0, a2))
                ps1[rel] = ps

            # ---- stage 2: beo = e * (hi+lo), re-split to bf16 hi/lo ----
            beo = {}
            for ri, (rel, plan) in enumerate((("i", plan_i), ("o", plan_o))):
                nch3 = plan.nch3
                ew_col = sm_sb[:, ri * nch3_i:ri * nch3_i + nch3]
                # hi+lo merge in ONE op: reduce over the h axis of the
                # [p, c, h, d] PSUM view
                ps4d = ps1[rel].rearrange("p (c h d) -> p c d h", h=2, d=4)
                bo = small.tile([P, nch3 * 4], F32, name=f"bo_{rel}")
                bo3 = bo.rearrange("p (c d) -> p c d", d=4)
                nc.vector.tensor_reduce(
                    bo3, ps4d, mybir.AxisListType.X, ADD)
                # e-scale + bf16 hi/lo split on GpSimd - frees the vector
                # engine (critical path) during this window
                ewb = ew_col.unsqueeze(2).to_broadcast([P, nch3, 4])
                nc.gpsimd.tensor_tensor(bo3, bo3, ewb, MUL)
                hl = small.tile([P, nch3 * 8], BF16, name=f"beo_{rel}")
                hl3 = hl.rearrange("p (c d) -> p c d", d=8)
                nc.gpsimd.tensor_copy(hl3[:, :, 0:4], bo3)
                nc.gpsimd.tensor_tensor(
                    hl3[:, :, 4:8], bo3, hl3[:, :, 0:4],
                    mybir.AluOpType.subtract)
                beo[rel] = hl

            # ---- stage 3: mi[n, d(hi,lo)] += onehot_dst^T @ beo --------
            # one 8-col MM per chunk (one LDWEIGHTS each - they dominate);
            # the hi+lo merge happens during eviction via tensor_reduce
            ps3 = {}
            for rel, plan in (("i", plan_i), ("o", plan_o)):
                nch3 = plan.nch3
                ps = ps3p.tile([P, 8], F32, name=f"ps3_{rel}", tag="ps3")
                hl3 = beo[rel].rearrange("p (c d) -> p c d", d=8)
                for ec in range(nch3):
                    nc.tensor.matmul(
                        ps[:],
                        t3_sb[rel][:, ec * P:(ec + 1) * P],
                        hl3[:, ec, :],
                        start=(ec == 0), stop=(ec == nch3 - 1))
                ps3[rel] = ps

            # ---- circuit: build M angles -------------------------------
            # pair layout: col 2w = m[w], col 2w+1 = m[w] + pi/2, wire-col
            # order w: [targets w1,w6,w10,w2,w5,w9 | controls w0,w7,...]
            # mi -> wire-cols {0,3,6,9} -> pair cols 0:24:6; mo -> 2:24:6;
            # X -> 4:24:6.
            m_ang = small.tile([P, 24], F32, name="m_ang")
            nc.vector.tensor_reduce(
                m_ang[:, 0:24:6],
                ps3["i"].rearrange("p (h d) -> p d h", h=2),
                mybir.AxisListType.X, ADD)
            nc.vector.tensor_reduce(
                m_ang[:, 2:24:6],
                ps3["o"].rearrange("p (h d) -> p d h", h=2),
                mybir.AxisListType.X, ADD)
            nc.vector.tensor_copy(
                m_ang[:, 4:24:6], sm_sb[:, sm_xk:sm_xk + 4])
            nc.vector.tensor_scalar(
                m_ang[:, 1:24:2], m_ang[:, 0:24:2], HPI, None, ADD)

            # range-reduce into [-pi, pi]: m2 = clamp(m - 2pi*rne(m/2pi))
            # via the fp32 magic-number trick: RN(x + 1.5*2^23) == rne(x)+M
            TWO_PI = float(2 * np.pi)
            t_z = small.tile([P, 24], F32, name="t_z")
            t_r = small.tile([P, 24], F32, name="t_r")
            m2 = small.tile([P, 24], F32, name="m2")
            nc.vector.scalar_tensor_tensor(
                t_z[:], m_ang[:], float(1.0 / TWO_PI), mtile[:], MUL, ADD)
            nc.vector.tensor_scalar(
                t_r[:], t_z[:], -RNE_M, None, ADD)
            nc.vector.scalar_tensor_tensor(
                m2[:], t_r[:], -TWO_PI, m_ang[:], MUL, ADD)
            nc.vector.tensor_scalar(
                m2[:], m2[:], PI, -PI,
                mybir.AluOpType.min, mybir.AluOpType.max)
            sxz = small.tile([P, 24], F32, name="sxz")
            nc.scalar.activation(sxz[:], m2[:],
                                 mybir.ActivationFunctionType.Sin)

            TT = nc.vector.tensor_tensor
            RED = nc.vector.tensor_reduce

            AX = mybir.AxisListType.X

            # sxz pairs: [p, wire-col (stride 2), {sin, cos} (stride 1)]
            sp = sxz.rearrange("p (t m) -> p t m", m=2)
            spb = sp[:, 0:6, :]      # target wires: (sx, sz) per col
            spa = sp[:, 6:12, :]     # control wires

            # ---- layer A ----------------------------------------------
            # ab[t, i] = ckA[i, t, 0]*sx_b[t] + ckA[i, t, 1]*sz_b[t]
            # az6[t] = ckZ6[t, 0]*sx_a[t] + ckZ6[t, 1]*sz_a[t]
            tmpA = small.tile([P, 36], F32, name="tmpA")
            ab = small.tile([P, 18], F32, name="ab")    # col = 3*t + i
            ckA = ckc(CK_A, 36).rearrange("p (i t m) -> p i t m", i=3, m=2)
            TT(tmpA.rearrange("p (i t m) -> p i t m", i=3, m=2),
               ckA, spb.unsqueeze(1).to_broadcast([P, 3, 6, 2]), MUL)
            RED(ab.rearrange("p (t i) -> p i t", t=6),
                tmpA.rearrange("p (i t m) -> p i t m", i=3, m=2), AX, ADD)
            tmpZ = small.tile([P, 12], F32, name="tmpZ")
            az6 = small.tile([P, 6], F32, name="az6")
            TT(tmpZ.rearrange("p (t m) -> p t m", m=2),
               ckc(CK_Z6, 12).rearrange("p (t m) -> p t m", m=2), spa, MUL)
            RED(az6[:], tmpZ.rearrange("p (t m) -> p t m", m=2), AX, ADD)
            # CNOT: scale y,z components by control z
            ab3 = ab.rearrange("p (t i) -> p t i", t=6)
            TT(ab3[:, :, 1:3], ab3[:, :, 1:3],
               az6.unsqueeze(2).to_broadcast([P, 6, 2]), MUL)

            # ---- layer B ----------------------------------------------
            # in: ab cols 0:9 = a-wires [w1,w6,w10], 9:18 = b [w2,w5,w9]
            # bb[t, i] = sum_j ckB[i, t, j] * ab_b[t, j]
            tmpB = small.tile([P, 27], F32, name="tmpB")
            bb = small.tile([P, 9], F32, name="bb")     # col = 3*t + i
            bv = ab[:, 9:18].rearrange("p (t j) -> p t j", t=3)
            ckB = ckc(CK_B, 27).rearrange("p (i t j) -> p i t j", i=3, j=3)
            TT(tmpB.rearrange("p (i t j) -> p i t j", i=3, j=3),
               ckB, bv.unsqueeze(1).to_broadcast([P, 3, 3, 3]), MUL)
            RED(bb.rearrange("p (t i) -> p i t", t=3),
                tmpB.rearrange("p (i t j) -> p i t j", i=3, j=3), AX, ADD)
            tmpZ3 = small.tile([P, 9], F32, name="tmpZ3")
            az3 = small.tile([P, 3], F32, name="az3")
            av = ab[:, 0:9].rearrange("p (t j) -> p t j", t=3)
            TT(tmpZ3.rearrange("p (t j) -> p t j", t=3),
               ckc(CK_Z3, 9).rearrange("p (t j) -> p t j", t=3), av, MUL)
            RED(az3[:], tmpZ3.rearrange("p (t j) -> p t j", t=3), AX, ADD)
            bb3 = bb.rearrange("p (t i) -> p t i", t=3)
            TT(bb3[:, :, 1:3], bb3[:, :, 1:3],
               az3.unsqueeze(2).to_broadcast([P, 3, 2]), MUL)

            # ---- layer C: blocks 9 then 10 -----------------------------
            # bb cols: 0:3 = w2 (x,y,z), 3:6 = w5, 6:9 = w9
            s9 = small.tile([P, 1], F32, name="s9")
            tr3 = small.tile([P, 3], F32, name="tr3")
            TT(tr3[:], ckc(CK_C18, 3), bb[:, 0:3], MUL)
            RED(s9[:], tr3[:], AX, ADD)
            w5 = small.tile([P, 3], F32, name="w5")
            tmp9 = small.tile([P, 9], F32, name="tmp9")
            ck19 = ckc(CK_C19, 9).rearrange("p (i j) -> p i j", i=3)
            TT(tmp9.rearrange("p (i j) -> p i j", i=3), ck19,
               bb[:, 3:6].unsqueeze(1).to_broadcast([P, 3, 3]), MUL)
            RED(w5[:], tmp9.rearrange("p (i j) -> p i j", i=3), AX, ADD)
            TT(w5[:, 1:3], w5[:, 1:3],
               s9.to_broadcast([P, 2]), MUL)
            s10 = small.tile([P, 1], F32, name="s10")
            tr3b = small.tile([P, 3], F32, name="tr3b")
            TT(tr3b[:], ckc(CK_C20, 3), w5[:], MUL)
            RED(s10[:], tr3b[:], AX, ADD)
            u = small.tile([P, 1], F32, name="u")
            tr3c = small.tile([P, 3], F32, name="tr3c")
            TT(tr3c[:], ckc(CK_C21, 3), bb[:, 6:9], MUL)
            RED(u[:], tr3c[:], AX, ADD)

            zf = small.tile([P, 1], F32, name="zf")
            TT(zf[:], s10[:], u[:], MUL)
            # result -> col 0 of a [128, 32] tile; 32x32 block transpose
            # scatters it to rows {0,32,64,96} x 32 cols -> contiguous store
            tr_out = small.tile([P, 32], F32, name="tr_out")
            nc.vector.tensor_scalar(
                tr_in[:, 0:1], zf[:], -PI, PI, MUL, ADD)
            nc.vector.transpose(tr_out[:], tr_in[:])
            nc.sync.dma_start(out[:], tr_out[0:P:32, :])

    return nc


_NC_CACHE = {}
_RUN_KWARGS = {}      # test harness can set e.g. {"trace": True}
_LAST_RESULTS = []    # BassKernelResults of the most recent run


def _get_nc(plan_i, plan_o):
    key = (plan_i.key(), plan_o.key())
    if _NC_CACHE.get("key") != key:
        nc = _build_nc(plan_i, plan_o)
        _split_multi_waits(nc)
        _NC_CACHE["key"] = key
        _NC_CACHE["nc"] = nc
    return _NC_CACHE["nc"]


def _split_hl(v):
    """fp32 -> (hi, lo) bf16 pair."""
    bf = ml_dtypes.bfloat16
    hi = v.astype(bf)
    lo = (v - hi.astype(np.float32)).astype(bf)
    return hi, lo


def kernel(X, e, Ri, Ro, theta):
    X = np.ascontiguousarray(np.asarray(X, np.float32))
    e = np.ascontiguousarray(np.asarray(e, np.float32))
    Ri = np.asarray(Ri, np.float32)
    Ro = np.asarray(Ro, np.float32)
    theta = np.asarray(theta, np.float32)
    bf = ml_dtypes.bfloat16

    idx_i = np.ascontiguousarray(Ri.argmax(axis=0))
    idx_o = np.ascontiguousarray(Ro.argmax(axis=0))

    plans, places = {}, {}
    for rel, (dst, src) in (("i", (idx_i, idx_o)), ("o", (idx_o, idx_i))):
        plan = _Plan(dst, src)
        plans[rel] = plan
        places[rel] = _route(dst, src, e, plan)

    ck1 = _pack_ck(theta)
    nch3_i, nch3_o = plans["i"].nch3, plans["o"].nch3
    sm_xk = nch3_i + nch3_o
    sm_w = sm_xk + 4 + CK_W

    # X hi/lo split, chunk-major, feature-permuted per rel
    xh, xl = _split_hl(X)          # [N, 4] each
    xhl_rel = {}
    for rel, perm in (("i", MI_PERM), ("o", MO_PERM)):
        a = np.zeros((P, NCH, 8), np.float32)
        a[:, :, 0:4] = xh.astype(np.float32)[:, perm].reshape(NCH, P, 4)\
            .transpose(1, 0, 2)
        a[:, :, 4:8] = xl.astype(np.float32)[:, perm].reshape(NCH, P, 4)\
            .transpose(1, 0, 2)
        xhl_rel[rel] = np.ascontiguousarray(
            a.reshape(P, NCH * 8).astype(bf))

    srcs = {"i": idx_o, "o": idx_i}
    dsts = {"i": idx_i, "o": idx_o}
    in_maps = []
    for k in range(N_CORES):
        im = {}
        sm = np.zeros((P, sm_w), np.float32)
        for ri, rel in enumerate(("i", "o")):
            plan, place = plans[rel], places[rel][k]
            src, dst = srcs[rel], dsts[rel]
            valid = place >= 0
            pe = place[valid]
            jj = np.nonzero(valid)[0]
            # stage-1 one-hot: [src_row_in_chunk, col]
            s1 = np.zeros((P, plan.s1w), bf)
            s1[src[pe] & 127, jj] = 1.0
            im[f"s1{rel}"] = s1
            # stage-3 one-hot: [e_in_chunk (partition), chunk*128 + own_node]
            t3 = np.zeros((P, plan.nch3 * P), bf)
            t3[jj & 127, (jj >> 7) * P + (dst[pe] & 127)] = 1.0
            im[f"t3{rel}"] = t3
            # e weights [p, chunk]
            ew = np.zeros(plan.s1w, np.float32)
            ew[jj] = e[pe]
            sm[:, ri * nch3_i:ri * nch3_i + plans[rel].nch3] = \
                ew.reshape(plan.nch3, P).T
        sm[:, sm_xk:sm_xk + 4] = X[k * P:(k + 1) * P][:, XK_PERM]
        sm[:, sm_xk + 4:] = ck1[None, :]
        comb = np.concatenate(
            [xhl_rel["i"].view(np.float32), xhl_rel["o"].view(np.float32),
             sm], axis=1)
        im["smalls"] = np.ascontiguousarray(comb)
        in_maps.append(im)

    nc = _get_nc(plans["i"], plans["o"])
    res = run_bass_kernel_spmd(nc, in_maps, core_ids=list(range(N_CORES)),
                               **_RUN_KWARGS)
    _LAST_RESULTS.clear()
    _LAST_RESULTS.append(res)
    return np.concatenate(
        [res.results[k]["out"].reshape(-1) for k in range(N_CORES)]
    ).astype(np.float32)


# revision 33
# speedup vs baseline: 1.0292x; 1.0292x over previous
"""Trainium2 Bass kernel for nn_NodeNet: GNN message passing + 12-qubit TTN circuit.

Math: the reference's statevector circuit contracts exactly to per-node
Bloch-vector chains (every CNOT block keeps only its target wire; the
measurement is <Z_9>; the circuit is a tree so alive wires stay in
product states). Per node the whole circuit is ~60 scalar ops.

The relation matrices Ri/Ro are one-hot per edge column (built with
.at[idx, arange(E)].set(1)), so the graph phase is a gather + weighted
scatter-add:  mi[n] = sum_{e: idx_i[e]=n} e[e] * X[idx_o[e]]  (mo swaps
the roles of idx_i/idx_o).

Sharding: edges are routed to the core that owns their DESTINATION node
chunk (128 nodes per core), separately for mi (by idx_i) and mo (by
idx_o).  Each core then computes mi/mo for its own nodes only -- no
cross-core reduction at all.  Within a core, edges are sorted by SOURCE
chunk so the gather one-hot only ever contracts over one 128-row X
chunk: layout = 8 full 128-col segments (first 128 edges per src chunk)
plus a 32-aligned overflow region.

Stage 1 (gather):  stationary = one-hot [128 src, seg cols], moving =
X chunk hi/lo bf16 [128, 8] -> PSUM [e-part, 8] per e-chunk.
Stage 2: hi+lo merge, scale by e, re-split to bf16 hi/lo (vector ops).
Stage 3 (scatter): stationary = dst one-hot [128 e, 128 n], moving =
beo [128, 8], accumulated over e-chunks -> PSUM [128 n, 8] which is
exactly the circuit's angle layout (features pre-permuted on host).

Precision: one-hots are exact in bf16; X and beo are carried as bf16
high+low splits (fp32-grade accuracy); all accumulation in fp32 PSUM.
"""

import hashlib

import ml_dtypes
import numpy as np

import bass_rust
import concourse.bass as bass
import concourse.mybir as mybir
import concourse.tile as tile
from concourse.bass_utils import run_bass_kernel_spmd

F32 = mybir.dt.float32
BF16 = mybir.dt.bfloat16
N_CORES = 8
N, E, D = 1024, 8192, 4
P = 128                  # partitions / nodes per core
NCH = N // P             # 8 node chunks

_BLOCKS = [(0, 1, (0, 1)), (2, 3, (3, 2)), (4, 5, (4, 5)), (6, 7, (7, 6)),
           (8, 9, (8, 9)), (10, 11, (11, 10)), (1, 2, (1, 2)), (5, 6, (6, 5)),
           (9, 10, (10, 9)), (2, 5, (2, 5)), (5, 9, (5, 9))]

# M-angle column layout (see baseline derivation):
# m cols 0:6 = layer-A target wires [w1, w6, w10, w2, w5, w9]
# m cols 6:12 = layer-A control wires [w0, w7, w11, w3, w4, w8]
# stride-3 view: col t of group g: {0,3,6,9}=mi, {1,4,7,10}=mo, {2,5,8,11}=X
A_BLOCKS = [0, 3, 5, 1, 2, 4]     # block idx per A-target col
B_BLOCKS = [6, 7, 8]              # b-cols [w2, w5, w9] <- a-cols [w1, w6, w10]
MI_PERM = [1, 2, 0, 3]            # mi feature cols in M stride-3 order
MO_PERM = [2, 1, 3, 0]            # mo feature cols
XK_PERM = [2, 1, 3, 0]            # X columns in M stride-3 order

# ---------------------------------------------------------------------------
# Host-side circuit-constant preparation (identical to proven baseline)
# ---------------------------------------------------------------------------

_PAULI = np.array([
    [[0, 1], [1, 0]],
    [[0, -1j], [1j, 0]],
    [[1, 0], [0, -1]],
], dtype=np.complex128)


def _rot_so3(p):
    """SO(3) Bloch rotation of Rot(phi, theta, omega) = RZ(om) RY(th) RZ(phi)."""
    phi, th, om = float(p[0]), float(p[1]), float(p[2])
    c, s = np.cos(th / 2), np.sin(th / 2)
    U = np.array([
        [np.exp(-0.5j * (phi + om)) * c, -np.exp(0.5j * (phi - om)) * s],
        [np.exp(-0.5j * (phi - om)) * s, np.exp(0.5j * (phi + om)) * c],
    ])
    R = np.empty((3, 3))
    for i in range(3):
        for j in range(3):
            R[i, j] = 0.5 * np.real(
                np.trace(_PAULI[i] @ U @ _PAULI[j] @ U.conj().T))
    return R


# circuit-constants column layout (offsets into the ck segment of smalls)
CK_A = 0          # layer A target coefs [(i*6+t)*2+m], m: 0=sin, 1=cos term
CK_Z6 = 36        # layer A control row2 [(t*2+m)]
CK_B = 48         # layer B target coefs [(i*3+t)*3+j]
CK_Z3 = 75        # layer B control row2 [(t*3+j)]
CK_C18 = 84       # R18 row 2 (block 9 control rot)
CK_C19 = 87       # R19 full 3x3 [i*3+j] (block 9 target rot)
CK_C20 = 96      # R20 row 2 (block 10 control rot)
CK_C21 = 99      # R21 row 2 (block 10 target rot)
CK_W = 102


def _pack_ck(theta):
    th = np.asarray(theta, np.float64)
    R = [_rot_so3(th[3 * k:3 * k + 3]) for k in range(23)]
    ck = np.zeros(CK_W, np.float64)

    for t, bidx in enumerate(A_BLOCKS):
        w1, w2, (c, tt) = _BLOCKS[bidx]
        k1, k2 = 2 * bidx, 2 * bidx + 1
        Rc = R[k1] if c == w1 else R[k2]
        Rt = R[k1] if tt == w1 else R[k2]
        for i in range(3):
            for m, j2 in enumerate((0, 2)):
                ck[CK_A + (i * 6 + t) * 2 + m] = Rt[i, j2]
        for m, j2 in enumerate((0, 2)):
            ck[CK_Z6 + t * 2 + m] = Rc[2, j2]

    for t, bidx in enumerate(B_BLOCKS):
        w1, w2, (c, tt) = _BLOCKS[bidx]
        k1, k2 = 2 * bidx, 2 * bidx + 1
        Rc = R[k1] if c == w1 else R[k2]
        Rt = R[k1] if tt == w1 else R[k2]
        for i in range(3):
            for j in range(3):
                ck[CK_B + (i * 3 + t) * 3 + j] = Rt[i, j]
        for j in range(3):
            ck[CK_Z3 + t * 3 + j] = Rc[2, j]

    # layer C: block 9 = (2,5,(2,5)): control rot R[18] (wire2), target R[19]
    #          block 10 = (5,9,(5,9)): control rot R[20] (wire5), target R[21]
    ck[CK_C19:CK_C19 + 9] = R[19].reshape(-1)
    ck[CK_C18:CK_C18 + 3] = R[18][2]
    ck[CK_C21:CK_C21 + 3] = R[21][2]
    ck[CK_C20:CK_C20 + 3] = R[20][2]
    return ck.astype(np.float32)


# ---------------------------------------------------------------------------
# Edge routing plan (host): data-dependent but deterministic per input set
# ---------------------------------------------------------------------------


class _Plan:
    """Shared-across-cores layout: per-src-chunk full segments (128 cols)
    plus 32-aligned overflow sub-segments, and the static MM interval list.
    """

    def __init__(self, idx_dst, idx_src):
        core = idx_dst >> 7
        chunk = idx_src >> 7
        cnt = np.bincount(core * NCH + chunk, minlength=64).reshape(8, NCH)
        ow = np.maximum(cnt - P, 0).max(axis=0)
        ow = (ow + 31) // 32 * 32                  # overflow width per chunk
        base_off = [c * P for c in range(NCH)]     # full segments
        of_off = []
        pos = NCH * P
        for c in range(NCH):
            of_off.append(pos)
            pos += int(ow[c])
        s1w = (pos + P - 1) // P * P               # pad to e-chunk multiple
        self.cnt, self.ow = cnt, ow
        self.base_off, self.of_off = base_off, of_off
        self.s1w = s1w
        self.nch3 = s1w // P
        # MM intervals: (col_start, width, src_chunk), split at 128 bounds
        iv = []
        for c in range(NCH):
            iv.append((base_off[c], P, c))
            if ow[c] > 0:
                # 32-col pieces keep every PSUM base/width combo legal
                a, w = of_off[c], int(ow[c])
                while w > 0:
                    take = 32
                    iv.append((a, take, c))
                    a += take
                    w -= take
        # cover trailing dead cols (all-zero one-hot -> writes zeros)
        end = max(a + w for a, w, _ in iv)
        while end < s1w:
            iv.append((end, 32, NCH - 1))
            end += 32
        self.intervals = tuple(sorted(iv))

    def key(self):
        return (self.s1w, self.intervals)


def _route(idx_dst, idx_src, ew, plan):
    """Per-core edge placement: returns (cols[core][j]->edge or -1)."""
    core = idx_dst >> 7
    chunk = idx_src >> 7
    order = np.lexsort((chunk, core))
    place = np.full((N_CORES, plan.s1w), -1, np.int64)
    for k in range(N_CORES):
        sel = order[core[order] == k]
        ch = chunk[sel]
        for c in range(NCH):
            seg = sel[ch == c]
            nfull = min(len(seg), P)
            place[k, plan.base_off[c]:plan.base_off[c] + nfull] = seg[:nfull]
            nov = len(seg) - nfull
            if nov > 0:
                o = plan.of_off[c]
                place[k, o:o + nov] = seg[nfull:]
    return place


# ---------------------------------------------------------------------------
# Walrus workaround: this build rejects >1 sync-wait per instruction
# ---------------------------------------------------------------------------


def _split_multi_waits(nc):
    for f in nc.m.functions:
        for bb in f.blocks:
            out = []
            for inst in bb.instructions:
                si = inst.sync_info
                if si is not None and si.on_wait and len(si.on_wait) > 1:
                    waits = list(si.on_wait)
                    for i, w in enumerate(waits[:-1]):
                        out.append(mybir.InstNoOp(
                            name=f"{inst.name}_wsplit{i}",
                            engine=inst.engine,
                            ins=[], outs=[],
                            sync_info=bass_rust.SyncInfo(
                                on_wait=[w], on_update=[]),
                        ))
                    inst.sync_info = bass_rust.SyncInfo(
                        on_wait=[waits[-1]], on_update=list(si.on_update))
                out.append(inst)
            bb.instructions = out


# ---------------------------------------------------------------------------
# Device kernel
# ---------------------------------------------------------------------------

# smalls layout: [ew_i(NCH3) | ew_o(NCH3) | xk(4) | ck(CK_W)]


def _build_nc(plan_i, plan_o):
    nc = bass.Bass("TRN2", target_bir_lowering=False, num_devices=N_CORES)

    nch3_i, nch3_o = plan_i.nch3, plan_o.nch3
    sm_xk = nch3_i + nch3_o
    sm_ck = sm_xk + 4
    sm_w = sm_ck + CK_W

    s1i_d = nc.declare_dram_parameter("s1i", [P, plan_i.s1w], BF16,
                                      isOutput=False)
    s1o_d = nc.declare_dram_parameter("s1o", [P, plan_o.s1w], BF16,
                                      isOutput=False)
    t3i_d = nc.declare_dram_parameter("t3i", [P, nch3_i * P], BF16,
                                      isOutput=False)
    t3o_d = nc.declare_dram_parameter("t3o", [P, nch3_o * P], BF16,
                                      isOutput=False)
    # one combined small tensor = one DMA (descriptor count is per-DMA);
    # cols 0:32 / 32:64 hold xhl_i / xhl_o bf16 pairs bit-packed in f32
    sm_d = nc.declare_dram_parameter("smalls", [P, 64 + sm_w], F32,
                                     isOutput=False)
    # [4, 32] so the store is 4 contiguous descriptors, not 128 4-byte ones
    # (the epilogue waits on store completion; scattered stores cost ~6us)
    out = nc.declare_dram_parameter("out", [4, 32], F32, isOutput=True)

    HPI = float(np.pi / 2)
    PI = float(np.pi)
    MUL = mybir.AluOpType.mult
    ADD = mybir.AluOpType.add

    with tile.TileContext(nc) as tc:
        with (
            tc.tile_pool(name="big", bufs=1) as big,
            tc.tile_pool(name="small", bufs=1) as small,
            tc.tile_pool(name="ps1", bufs=2, space="PSUM") as ps1p,
            tc.tile_pool(name="ps3", bufs=2, space="PSUM") as ps3p,
        ):
            # ---- inputs (HWDGE queues only; SWDGE drains cost us) ------
            # order: stage-1-i gates everything, so s1_i issues first
            s1_sb, t3_sb = {}, {}
            s1_sb["i"] = big.tile([P, plan_i.s1w], BF16, name="s1_i",
                                  tag="s1_i")
            nc.sync.dma_start(s1_sb["i"][:], s1i_d[:])
            comb = small.tile([P, 64 + sm_w], F32, name="comb")
            nc.scalar.dma_start(comb[:], sm_d[:])
            s1_sb["o"] = big.tile([P, plan_o.s1w], BF16, name="s1_o",
                                  tag="s1_o")
            nc.sync.dma_start(s1_sb["o"][:], s1o_d[:])
            t3_sb["i"] = big.tile([P, plan_i.nch3 * P], BF16, name="t3_i",
                                  tag="t3_i")
            nc.scalar.dma_start(t3_sb["i"][:], t3i_d[:])
            t3_sb["o"] = big.tile([P, plan_o.nch3 * P], BF16, name="t3_o",
                                  tag="t3_o")
            nc.scalar.dma_start(t3_sb["o"][:], t3o_d[:])
            xhl = {"i": comb[:, 0:32].bitcast(BF16),
                   "o": comb[:, 32:64].bitcast(BF16)}
            sm_sb = comb[:, 64:]

            def ckc(off, n=1):
                return sm_sb[:, sm_ck + off:sm_ck + off + n]

            # preload the ACT Sin table set while DMAs stream
            warm = small.tile([P, 1], F32, name="warm")
            nc.vector.memset(warm[:], 0.0)
            nc.scalar.activation(warm[:], warm[:],
                                 mybir.ActivationFunctionType.Sin)
            tr_in = small.tile([P, 32], F32, name="tr_in")
            nc.vector.memset(tr_in[:], 0.0)
            # rne magic constant tile: adding 1.5*2^23 forces fp32 RNE to
            # integer; subtracting it back recovers rne(x) exactly
            RNE_M = float(1.5 * 2 ** 23)
            mtile = small.tile([P, 24], F32, name="mtile")
            nc.vector.memset(mtile[:], RNE_M)

            # ---- stage 1: bo[e, d(hi,lo)] = onehot_src^T @ Xsplit ------
            ps1 = {}
            for rel, plan in (("i", plan_i), ("o", plan_o)):
                ps = ps1p.tile([P, plan.nch3 * 8], F32, name=f"ps1_{rel}",
                               tag="ps1")
                for (a, w, c) in plan.intervals:
                    ec, a2 = a // P, a % P
                    nc.tensor.matmul(
                        ps[a2:a2 + w, ec * 8:ec * 8 + 8],
                        s1_sb[rel][:, a:a + w],
                        xhl[rel][:, c * 8:(c + 1) * 8],
                        start=True, stop=True,
                        tile_position=(0, a2))
                ps1[rel] = ps

            # ---- stage 2: beo = e * (hi+lo), re-split to bf16 hi/lo ----
            beo = {}
            for ri, (rel, plan) in enumerate((("i", plan_i), ("o", plan_o))):
                nch3 = plan.nch3
                ew_col = sm_sb[:, ri * nch3_i:ri * nch3_i + nch3]
                # hi+lo merge in ONE op: reduce over the h axis of the
                # [p, c, h, d] PSUM view
                ps4d = ps1[rel].rearrange("p (c h d) -> p c d h", h=2, d=4)
                bo = small.tile([P, nch3 * 4], F32, name=f"bo_{rel}")
                bo3 = bo.rearrange("p (c d) -> p c d", d=4)
                nc.vector.tensor_reduce(
                    bo3, ps4d, mybir.AxisListType.X, ADD)
                ewb = ew_col.unsqueeze(2).to_broadcast([P, nch3, 4])
                nc.vector.tensor_tensor(bo3, bo3, ewb, MUL)
                hl = small.tile([P, nch3 * 8], BF16, name=f"beo_{rel}")
                hl3 = hl.rearrange("p (c d) -> p c d", d=8)
                nc.vector.tensor_copy(hl3[:, :, 0:4], bo3)
                nc.vector.scalar_tensor_tensor(
                    hl3[:, :, 4:8], hl3[:, :, 0:4], -1.0, bo3, MUL, ADD)
                beo[rel] = hl

            # ---- stage 3: mi[n, d(hi,lo)] += onehot_dst^T @ beo --------
            # one 8-col MM per chunk (one LDWEIGHTS each - they dominate);
            # the hi+lo merge happens during eviction via tensor_reduce
            ps3 = {}
            for rel, plan in (("i", plan_i), ("o", plan_o)):
                nch3 = plan.nch3
                ps = ps3p.tile([P, 8], F32, name=f"ps3_{rel}", tag="ps3")
                hl3 = beo[rel].rearrange("p (c d) -> p c d", d=8)
                for ec in range(nch3):
                    nc.tensor.matmul(
                        ps[:],
                        t3_sb[rel][:, ec * P:(ec + 1) * P],
                        hl3[:, ec, :],
                        start=(ec == 0), stop=(ec == nch3 - 1))
                ps3[rel] = ps

            # ---- circuit: build M angles -------------------------------
            # pair layout: col 2w = m[w], col 2w+1 = m[w] + pi/2, wire-col
            # order w: [targets w1,w6,w10,w2,w5,w9 | controls w0,w7,...]
            # mi -> wire-cols {0,3,6,9} -> pair cols 0:24:6; mo -> 2:24:6;
            # X -> 4:24:6.
            m_ang = small.tile([P, 24], F32, name="m_ang")
            nc.vector.tensor_reduce(
                m_ang[:, 0:24:6],
                ps3["i"].rearrange("p (h d) -> p d h", h=2),
                mybir.AxisListType.X, ADD)
            nc.vector.tensor_reduce(
                m_ang[:, 2:24:6],
                ps3["o"].rearrange("p (h d) -> p d h", h=2),
                mybir.AxisListType.X, ADD)
            nc.vector.tensor_copy(
                m_ang[:, 4:24:6], sm_sb[:, sm_xk:sm_xk + 4])
            nc.vector.tensor_scalar(
                m_ang[:, 1:24:2], m_ang[:, 0:24:2], HPI, None, ADD)

            # range-reduce into [-pi, pi]: m2 = clamp(m - 2pi*rne(m/2pi))
            # via the fp32 magic-number trick: RN(x + 1.5*2^23) == rne(x)+M
            TWO_PI = float(2 * np.pi)
            t_z = small.tile([P, 24], F32, name="t_z")
            t_r = small.tile([P, 24], F32, name="t_r")
            m2 = small.tile([P, 24], F32, name="m2")
            nc.vector.scalar_tensor_tensor(
                t_z[:], m_ang[:], float(1.0 / TWO_PI), mtile[:], MUL, ADD)
            nc.vector.tensor_scalar(
                t_r[:], t_z[:], -RNE_M, None, ADD)
            nc.vector.scalar_tensor_tensor(
                m2[:], t_r[:], -TWO_PI, m_ang[:], MUL, ADD)
            nc.vector.tensor_scalar(
                m2[:], m2[:], PI, -PI,
                mybir.AluOpType.min, mybir.AluOpType.max)
            sxz = small.tile([P, 24], F32, name="sxz")
            nc.scalar.activation(sxz[:], m2[:],
                                 mybir.ActivationFunctionType.Sin)

            TT = nc.vector.tensor_tensor
            RED = nc.vector.tensor_reduce

            AX = mybir.AxisListType.X

            # sxz pairs: [p, wire-col (stride 2), {sin, cos} (stride 1)]
            sp = sxz.rearrange("p (t m) -> p t m", m=2)
            spb = sp[:, 0:6, :]      # target wires: (sx, sz) per col
            spa = sp[:, 6:12, :]     # control wires

            # ---- layer A ----------------------------------------------
            # ab[t, i] = ckA[i, t, 0]*sx_b[t] + ckA[i, t, 1]*sz_b[t]
            # az6[t] = ckZ6[t, 0]*sx_a[t] + ckZ6[t, 1]*sz_a[t]
            tmpA = small.tile([P, 36], F32, name="tmpA")
            ab = small.tile([P, 18], F32, name="ab")    # col = 3*t + i
            ckA = ckc(CK_A, 36).rearrange("p (i t m) -> p i t m", i=3, m=2)
            TT(tmpA.rearrange("p (i t m) -> p i t m", i=3, m=2),
               ckA, spb.unsqueeze(1).to_broadcast([P, 3, 6, 2]), MUL)
            RED(ab.rearrange("p (t i) -> p i t", t=6),
                tmpA.rearrange("p (i t m) -> p i t m", i=3, m=2), AX, ADD)
            tmpZ = small.tile([P, 12], F32, name="tmpZ")
            az6 = small.tile([P, 6], F32, name="az6")
            TT(tmpZ.rearrange("p (t m) -> p t m", m=2),
               ckc(CK_Z6, 12).rearrange("p (t m) -> p t m", m=2), spa, MUL)
            RED(az6[:], tmpZ.rearrange("p (t m) -> p t m", m=2), AX, ADD)
            # CNOT: scale y,z components by control z
            ab3 = ab.rearrange("p (t i) -> p t i", t=6)
            TT(ab3[:, :, 1:3], ab3[:, :, 1:3],
               az6.unsqueeze(2).to_broadcast([P, 6, 2]), MUL)

            # ---- layer B ----------------------------------------------
            # in: ab cols 0:9 = a-wires [w1,w6,w10], 9:18 = b [w2,w5,w9]
            # bb[t, i] = sum_j ckB[i, t, j] * ab_b[t, j]
            tmpB = small.tile([P, 27], F32, name="tmpB")
            bb = small.tile([P, 9], F32, name="bb")     # col = 3*t + i
            bv = ab[:, 9:18].rearrange("p (t j) -> p t j", t=3)
            ckB = ckc(CK_B, 27).rearrange("p (i t j) -> p i t j", i=3, j=3)
            TT(tmpB.rearrange("p (i t j) -> p i t j", i=3, j=3),
               ckB, bv.unsqueeze(1).to_broadcast([P, 3, 3, 3]), MUL)
            RED(bb.rearrange("p (t i) -> p i t", t=3),
                tmpB.rearrange("p (i t j) -> p i t j", i=3, j=3), AX, ADD)
            tmpZ3 = small.tile([P, 9], F32, name="tmpZ3")
            az3 = small.tile([P, 3], F32, name="az3")
            av = ab[:, 0:9].rearrange("p (t j) -> p t j", t=3)
            TT(tmpZ3.rearrange("p (t j) -> p t j", t=3),
               ckc(CK_Z3, 9).rearrange("p (t j) -> p t j", t=3), av, MUL)
            RED(az3[:], tmpZ3.rearrange("p (t j) -> p t j", t=3), AX, ADD)
            bb3 = bb.rearrange("p (t i) -> p t i", t=3)
            TT(bb3[:, :, 1:3], bb3[:, :, 1:3],
               az3.unsqueeze(2).to_broadcast([P, 3, 2]), MUL)

            # ---- layer C: blocks 9 then 10 -----------------------------
            # bb cols: 0:3 = w2 (x,y,z), 3:6 = w5, 6:9 = w9
            s9 = small.tile([P, 1], F32, name="s9")
            tr3 = small.tile([P, 3], F32, name="tr3")
            TT(tr3[:], ckc(CK_C18, 3), bb[:, 0:3], MUL)
            RED(s9[:], tr3[:], AX, ADD)
            w5 = small.tile([P, 3], F32, name="w5")
            tmp9 = small.tile([P, 9], F32, name="tmp9")
            ck19 = ckc(CK_C19, 9).rearrange("p (i j) -> p i j", i=3)
            TT(tmp9.rearrange("p (i j) -> p i j", i=3), ck19,
               bb[:, 3:6].unsqueeze(1).to_broadcast([P, 3, 3]), MUL)
            RED(w5[:], tmp9.rearrange("p (i j) -> p i j", i=3), AX, ADD)
            TT(w5[:, 1:3], w5[:, 1:3],
               s9.to_broadcast([P, 2]), MUL)
            s10 = small.tile([P, 1], F32, name="s10")
            tr3b = small.tile([P, 3], F32, name="tr3b")
            TT(tr3b[:], ckc(CK_C20, 3), w5[:], MUL)
            RED(s10[:], tr3b[:], AX, ADD)
            u = small.tile([P, 1], F32, name="u")
            tr3c = small.tile([P, 3], F32, name="tr3c")
            TT(tr3c[:], ckc(CK_C21, 3), bb[:, 6:9], MUL)
            RED(u[:], tr3c[:], AX, ADD)

            zf = small.tile([P, 1], F32, name="zf")
            TT(zf[:], s10[:], u[:], MUL)
            # result -> col 0 of a [128, 32] tile; 32x32 block transpose
            # scatters it to rows {0,32,64,96} x 32 cols -> contiguous store
            tr_out = small.tile([P, 32], F32, name="tr_out")
            nc.vector.tensor_scalar(
                tr_in[:, 0:1], zf[:], -PI, PI, MUL, ADD)
            nc.vector.transpose(tr_out[:], tr_in[:])
            nc.sync.dma_start(out[:], tr_out[0:P:32, :])

    return nc


_NC_CACHE = {}
_RUN_KWARGS = {}      # test harness can set e.g. {"trace": True}
_LAST_RESULTS = []    # BassKernelResults of the most recent run


def _get_nc(plan_i, plan_o):
    key = (plan_i.key(), plan_o.key())
    if _NC_CACHE.get("key") != key:
        nc = _build_nc(plan_i, plan_o)
        _split_multi_waits(nc)
        _NC_CACHE["key"] = key
        _NC_CACHE["nc"] = nc
    return _NC_CACHE["nc"]


def _split_hl(v):
    """fp32 -> (hi, lo) bf16 pair."""
    bf = ml_dtypes.bfloat16
    hi = v.astype(bf)
    lo = (v - hi.astype(np.float32)).astype(bf)
    return hi, lo


def kernel(X, e, Ri, Ro, theta):
    X = np.ascontiguousarray(np.asarray(X, np.float32))
    e = np.ascontiguousarray(np.asarray(e, np.float32))
    Ri = np.asarray(Ri, np.float32)
    Ro = np.asarray(Ro, np.float32)
    theta = np.asarray(theta, np.float32)
    bf = ml_dtypes.bfloat16

    idx_i = np.ascontiguousarray(Ri.argmax(axis=0))
    idx_o = np.ascontiguousarray(Ro.argmax(axis=0))

    plans, places = {}, {}
    for rel, (dst, src) in (("i", (idx_i, idx_o)), ("o", (idx_o, idx_i))):
        plan = _Plan(dst, src)
        plans[rel] = plan
        places[rel] = _route(dst, src, e, plan)

    ck1 = _pack_ck(theta)
    nch3_i, nch3_o = plans["i"].nch3, plans["o"].nch3
    sm_xk = nch3_i + nch3_o
    sm_w = sm_xk + 4 + CK_W

    # X hi/lo split, chunk-major, feature-permuted per rel
    xh, xl = _split_hl(X)          # [N, 4] each
    xhl_rel = {}
    for rel, perm in (("i", MI_PERM), ("o", MO_PERM)):
        a = np.zeros((P, NCH, 8), np.float32)
        a[:, :, 0:4] = xh.astype(np.float32)[:, perm].reshape(NCH, P, 4)\
            .transpose(1, 0, 2)
        a[:, :, 4:8] = xl.astype(np.float32)[:, perm].reshape(NCH, P, 4)\
            .transpose(1, 0, 2)
        xhl_rel[rel] = np.ascontiguousarray(
            a.reshape(P, NCH * 8).astype(bf))

    srcs = {"i": idx_o, "o": idx_i}
    dsts = {"i": idx_i, "o": idx_o}
    in_maps = []
    for k in range(N_CORES):
        im = {}
        sm = np.zeros((P, sm_w), np.float32)
        for ri, rel in enumerate(("i", "o")):
            plan, place = plans[rel], places[rel][k]
            src, dst = srcs[rel], dsts[rel]
            valid = place >= 0
            pe = place[valid]
            jj = np.nonzero(valid)[0]
            # stage-1 one-hot: [src_row_in_chunk, col]
            s1 = np.zeros((P, plan.s1w), bf)
            s1[src[pe] & 127, jj] = 1.0
            im[f"s1{rel}"] = s1
            # stage-3 one-hot: [e_in_chunk (partition), chunk*128 + own_node]
            t3 = np.zeros((P, plan.nch3 * P), bf)
            t3[jj & 127, (jj >> 7) * P + (dst[pe] & 127)] = 1.0
            im[f"t3{rel}"] = t3
            # e weights [p, chunk]
            ew = np.zeros(plan.s1w, np.float32)
            ew[jj] = e[pe]
            sm[:, ri * nch3_i:ri * nch3_i + plans[rel].nch3] = \
                ew.reshape(plan.nch3, P).T
        sm[:, sm_xk:sm_xk + 4] = X[k * P:(k + 1) * P][:, XK_PERM]
        sm[:, sm_xk + 4:] = ck1[None, :]
        comb = np.concatenate(
            [xhl_rel["i"].view(np.float32), xhl_rel["o"].view(np.float32),
             sm], axis=1)
        im["smalls"] = np.ascontiguousarray(comb)
        in_maps.append(im)

    nc = _get_nc(plans["i"], plans["o"])
    res = run_bass_kernel_spmd(nc, in_maps, core_ids=list(range(N_CORES)),
                               **_RUN_KWARGS)
    _LAST_RESULTS.clear()
    _LAST_RESULTS.append(res)
    return np.concatenate(
        [res.results[k]["out"].reshape(-1) for k in range(N_CORES)]
    ).astype(np.float32)
